# revision 18
# baseline (speedup 1.0000x reference)
"""Trainium2 Bass kernel for nn_Attention_50173807952647.

GQA attention block: qkv projections + partial interleaved RoPE + softmax
attention + output projection, fp32 inputs/outputs.

Sharding: 8 cores; core d owns kv-head d and query heads {2d, 2d+1} for all
4 batches (head/tensor parallel per the GQA grouping). Each core computes a
partial output (its heads' contribution through Wo); host sums partials + bias.

v3: bf16 datapath + XBAR DMA-transpose x^T + fp8e4 DoubleRow attnV +
filler-interleaved emission.
  - Host converts x/weights to bf16; x^T lands in SBUF directly via
    dma_start_transpose (own contiguous [128,512] tile per transfer).
  - Scores: K=64 bf16 matmuls; k^T duplicated once per batch into
    partitions 64:128 so the head-odd matmul pairs with qT[64:128].
  - exp on ACT goes PSUM -> fp8e4 SBUF, pairs of k-tiles share one
    [128,2,1024] tile; attnV contracts both tiles of a pair in a single
    DoubleRow matmul (2 fp8 MACs/cell/cycle). v packed [kt, 80] with the
    denominator ones-column at offset 64.
  - PE queue is fed via a filler generator inside the attention k-tile
    loop: next chunk's projection matmuls first, previous block's Wo
    matmuls after (their oT needs the normalize chain to finish). This
    keeps the PE busy through the exp-gated inner loop (psA double
    buffering caps PE run-ahead at the ACT cadence).
"""

import sys

import numpy as np

HEADS = 16
KV_HEADS = 8
DIM_HEAD = 64
ROT_DIM = 32
SCALE = DIM_HEAD ** -0.5
B, N, DIM = 4, 2048, 1024
N_CORES = 8
T = B * N  # 8192 tokens
CHUNK = 512  # projection chunk (tokens)
QB = 512  # attention query block
FP8_ATTNV = False

_BUILT = {}


def _ensure_path():
    for p in ("/opt/trn_rl_repo",):
        if p not in sys.path:
            sys.path.insert(0, p)


def _to_bf16(a):
    import ml_dtypes
    return np.ascontiguousarray(np.asarray(a, np.float32).astype(ml_dtypes.bfloat16))


def _rope_tables():
    """cos/sin tables [128, N] for the transposed [hd, t] layout.

    Row r (hd index within a core's 128 q-rows): head-local d = r % 64.
    d < ROT_DIM: cos(t * inv_freq[d//2]); sin with rotate-half sign folded
    (-sin on even d, +sin on odd d). Elsewhere cos=1, sin=0 so a single
    full-width mul+add applies RoPE only where it belongs.
    """
    inv_freq = 1.0 / (10000.0 ** (np.arange(0, ROT_DIM, 2, dtype=np.float64) / ROT_DIM))
    t = np.arange(N, dtype=np.float64)
    freqs = t[None, :] * inv_freq[:, None]  # [16, N]
    cos = np.ones((128, N), dtype=np.float64)
    sin = np.zeros((128, N), dtype=np.float64)
    for r in range(128):
        d = r % 64
        if d < ROT_DIM:
            f = freqs[d // 2]
            cos[r] = np.cos(f)
            sin[r] = (-1.0 if d % 2 == 0 else 1.0) * np.sin(f)
    return cos.astype(np.float32), sin.astype(np.float32)


def _build(debug=False):
    if ("nc", debug) in _BUILT:
        return _BUILT[("nc", debug)]
    _ensure_path()
    import concourse.bass as bass  # noqa: F401
    import concourse.mybir as mybir
    import concourse.tile as tile
    from concourse import bacc
    from concourse.masks import make_identity

    dt = mybir.dt
    f32, bf16 = dt.float32, dt.bfloat16
    f8 = dt.float8e4
    edt = f8 if FP8_ATTNV else bf16
    AF = mybir.ActivationFunctionType
    OP = mybir.AluOpType
    PM = mybir.MatmulPerfMode

    nc = bacc.Bacc("TRN2", target_bir_lowering=False, debug=False)

    x_in = nc.dram_tensor("x", [B, N, DIM], bf16, kind="ExternalInput").ap()
    wq_in = nc.dram_tensor("wq", [DIM, 128], bf16, kind="ExternalInput").ap()
    wkv_in = nc.dram_tensor("wkv", [DIM, 128], bf16, kind="ExternalInput").ap()
    wo_in = nc.dram_tensor("wo", [128, DIM], bf16, kind="ExternalInput").ap()
    cos_in = nc.dram_tensor("cos_t", [128, N], f32, kind="ExternalInput").ap()
    sin_in = nc.dram_tensor("sin_t", [128, N], f32, kind="ExternalInput").ap()
    out_d = nc.dram_tensor("out", [T, DIM], f32, kind="ExternalOutput").ap()
    if debug:
        dbg_qT = nc.dram_tensor("dbg_qT", [128, N], bf16, kind="ExternalOutput").ap()
        dbg_kT = nc.dram_tensor("dbg_kT", [128, N], bf16, kind="ExternalOutput").ap()
        dbg_v = nc.dram_tensor("dbg_v", [128, (N // 128) * 80], edt, kind="ExternalOutput").ap()
        dbg_e = nc.dram_tensor("dbg_e", [128, 2048], edt, kind="ExternalOutput").ap()
        dbg_oT = nc.dram_tensor("dbg_oT", [128, QB], bf16, kind="ExternalOutput").ap()
        dbg_rec = nc.dram_tensor("dbg_rec", [1, 1024], f32, kind="ExternalOutput").ap()

    NCH = N // CHUNK  # chunks per batch
    NQB = N // QB  # q blocks per batch
    NKT = N // 128  # key tiles per batch
    NPR = NKT // 2  # key tile pairs
    pair_mask = []
    for i in range(16):
        pair_mask += [2 * i + 1, 2 * i]

    with tile.TileContext(nc) as tc:
        with (
            tc.tile_pool(name="const", bufs=1) as constp,
            tc.tile_pool(name="perbatch", bufs=2) as batchp,
            tc.tile_pool(name="xt", bufs=4) as xtp,
            tc.tile_pool(name="rope", bufs=6) as ropep,
            tc.tile_pool(name="sm", bufs=2) as smp,
            tc.tile_pool(name="exp", bufs=3) as expp,
            tc.tile_pool(name="osb", bufs=4) as osbp,
            tc.tile_pool(name="outsb", bufs=3) as outsbp,
            tc.tile_pool(name="psA", bufs=2, space="PSUM") as psA,
            tc.tile_pool(name="psB", bufs=1, space="PSUM") as psB,
            tc.tile_pool(name="psC", bufs=2, space="PSUM") as psC,
        ):
            ident = constp.tile([128, 128], bf16)
            make_identity(nc, ident[:])
            wq_sb = constp.tile([128, 8 * 128], bf16, tag="wq")
            wkv_sb = constp.tile([128, 8 * 128], bf16, tag="wkv")
            for et in range(8):
                nc.sync.dma_start(wq_sb[:, et * 128:(et + 1) * 128],
                                  wq_in[et * 128:(et + 1) * 128, :])
                nc.sync.dma_start(wkv_sb[:, et * 128:(et + 1) * 128],
                                  wkv_in[et * 128:(et + 1) * 128, :])
            wo_sb = constp.tile([128, DIM], bf16, tag="wo")
            nc.sync.dma_start(wo_sb[:], wo_in[:])
            ebias = constp.tile([128, 1], f32, tag="ebias")
            nc.vector.memset(ebias[:], -2.0 if FP8_ATTNV else 0.0)
            cos_sb = constp.tile([128, N], f32, tag="cos")
            sin_sb = constp.tile([128, N], f32, tag="sin")
            nc.sync.dma_start(cos_sb[:], cos_in[:])
            nc.sync.dma_start(sin_sb[:], sin_in[:])

            def load_xT(b):
                """DMA-transpose a whole batch of x into 8 separate
                [128, N] bf16 tiles (dst of the XBAR transpose must be
                contiguous). Issued a full batch ahead of use so the
                projection filler never waits on DMA arrival."""
                xts = []
                for et in range(8):
                    xte = xtp.tile([128, N], bf16, tag=f"xT{et}")
                    nc.sync.dma_start_transpose(
                        xte[:], x_in[b, :, et * 128:(et + 1) * 128])
                    xts.append(xte)
                return xts

            def load_xT_chunk(b, c):
                """Chunk-sized x^T transposes for the prologue batch (start
                projecting after ~1MB instead of the whole batch)."""
                xts = []
                for et in range(8):
                    xte = xtp.tile([128, CHUNK], bf16, tag=f"xTc{et}")
                    nc.sync.dma_start_transpose(
                        xte[:], x_in[b, c * CHUNK:(c + 1) * CHUNK,
                                     et * 128:(et + 1) * 128])
                    xts.append(xte)
                return xts

            def proj_fill(b, c, tiles, xap):
                """Generator: projection matmuls + rope for chunk c of batch
                b, yielded in PE-sized units so attn_core can interleave.
                xap(et) -> rhs AP of x^T chunk columns for lane et."""
                qT, kT, v_sb = tiles
                cs = slice(c * CHUNK, (c + 1) * CHUNK)
                qps = psC.tile([128, 512], f32, tag="ps_small")
                for et in range(8):
                    nc.tensor.matmul(qps[:],
                                     wq_sb[:, et * 128:(et + 1) * 128],
                                     xap(et),
                                     start=(et == 0), stop=(et == 7))
                    if et % 2 == 1:
                        yield
                kvps = psC.tile([128, 512], f32, tag="ps_small")
                for et in range(8):
                    nc.tensor.matmul(kvps[:],
                                     wkv_sb[:, et * 128:(et + 1) * 128],
                                     xap(et),
                                     start=(et == 0), stop=(et == 7))
                    if et % 2 == 1:
                        yield
                # rope epilogue: q (DVE only)
                shq = ropep.tile([128, CHUNK], f32, tag="rope")
                nc.vector.stream_shuffle(shq[:], qps[:], pair_mask)
                t1q = ropep.tile([128, CHUNK], f32, tag="rope")
                nc.vector.tensor_tensor(t1q[:], qps[:], cos_sb[:, cs], op=OP.mult)
                t2q = ropep.tile([128, CHUNK], f32, tag="rope")
                nc.vector.tensor_tensor(t2q[:], shq[:], sin_sb[:, cs], op=OP.mult)
                nc.vector.tensor_tensor(qT[:, cs], t1q[:], t2q[:], op=OP.add)
                yield
                # rope epilogue: k -> kT rows 0:64 (DVE only)
                shk = ropep.tile([32, CHUNK], f32, tag="rope")
                nc.vector.stream_shuffle(shk[:], kvps[0:32, :], pair_mask)
                t1k = ropep.tile([64, CHUNK], f32, tag="rope")
                nc.vector.tensor_tensor(t1k[:], kvps[0:64, :], cos_sb[0:64, cs],
                                        op=OP.mult)
                t2k = ropep.tile([32, CHUNK], f32, tag="rope")
                nc.vector.tensor_tensor(t2k[:], shk[:], sin_sb[0:32, cs], op=OP.mult)
                nc.vector.tensor_tensor(kT[0:32, cs], t1k[0:32, :], t2k[:], op=OP.add)
                nc.vector.tensor_copy(kT[32:64, cs], t1k[32:64, :])
                # duplicate k^T into partitions 64:128 so the head-odd score
                # matmul can pair with qT[64:128] (matmul needs equal base
                # partitions)
                nc.sync.dma_start(kT[64:128, cs], kT[0:64, cs])
                yield
                # v staging copy (DVE), then PE transposes + pack
                vts = ropep.tile([64, CHUNK], bf16, tag="ropev")
                nc.vector.tensor_copy(vts[:], kvps[64:128, :])
                yield
                vtp = psC.tile([128, 512], bf16, tag="ps_small")
                for st in range(4):
                    nc.tensor.transpose(vtp[:, st * 128: st * 128 + 64],
                                        vts[:, st * 128:(st + 1) * 128],
                                        ident[0:64, 0:64])
                yield
                v3 = v_sb[:].rearrange("p (kt c) -> p kt c", c=80)
                for st in range(4):
                    kt = c * 4 + st
                    nc.vector.tensor_copy(v3[:, kt, 0:64],
                                          vtp[:, st * 128: st * 128 + 64])
                yield

            def wo_fill(b, qb, oT):
                """Generator: previous block's out-projection, one matmul +
                staging copy + DMA per unit."""
                for ts in range(4):
                    for eh in range(2):
                        po = psC.tile([128, 512], f32, tag="ps_small")
                        nc.tensor.matmul(po[:],
                                         oT[:, ts * 128:(ts + 1) * 128],
                                         wo_sb[:, eh * 512:(eh + 1) * 512],
                                         start=True, stop=True)
                        ob = outsbp.tile([128, 512], f32, tag="ob")
                        nc.vector.tensor_copy(ob[:], po[:])
                        r0 = b * N + qb * QB + ts * 128
                        nc.sync.dma_start(
                            out_d[r0:r0 + 128, eh * 512:(eh + 1) * 512], ob[:])
                        yield

            def attn_core(b, qb, tiles, filler):
                qT, kT, v_sb = tiles
                v3 = v_sb[:].rearrange("p (kt c) -> p kt c", c=80)
                qs = slice(qb * QB, (qb + 1) * QB)
                ops_t = psB.tile([65, 1024], f32, tag="ps_o")

                def attnv(j, last):
                    if FP8_ATTNV:
                        nc.tensor.matmul(ops_t[:, 0:512],
                                         v3[:, 2 * j:2 * j + 2, 0:65],
                                         e2s[j][:, :, 0:512],
                                         start=(j == 0), stop=last,
                                         perf_mode=PM.DoubleRow)
                        nc.tensor.matmul(ops_t[:, 512:1024],
                                         v3[:, 2 * j:2 * j + 2, 0:65],
                                         e2s[j][:, :, 512:1024],
                                         start=(j == 0), stop=last,
                                         perf_mode=PM.DoubleRow)
                    else:
                        for h in range(2):
                            for kt in (2 * j, 2 * j + 1):
                                nc.tensor.matmul(
                                    ops_t[:, h * 512:(h + 1) * 512],
                                    v3[:, kt, 0:65],
                                    e2s[j][:, kt - 2 * j, h * 512:(h + 1) * 512],
                                    start=(kt == 0 and j == 0),
                                    stop=(last and kt == 2 * j + 1))

                e2s = []
                for pr in range(NPR):
                    e2 = expp.tile([128, 2, 1024], edt, tag="e")
                    for half in range(2):
                        kt = 2 * pr + half
                        ks = slice(kt * 128, (kt + 1) * 128)
                        sps = psA.tile([128, 1024], f32, tag="ps_big")
                        nc.tensor.matmul(sps[:, 0:512], kT[0:64, ks],
                                         qT[0:64, qs], start=True, stop=True)
                        nc.tensor.matmul(sps[:, 512:1024], kT[64:128, ks],
                                         qT[64:128, qs], start=True, stop=True)
                        # bias -2 keeps exp below the fp8e4 max (~240) for
                        # up to ~7.5-sigma scores; softmax shift-invariance
                        # cancels it exactly in O/den
                        nc.scalar.activation(e2[:, half, :], sps[:],
                                             AF.Exp, scale=SCALE,
                                             bias=ebias[:])
                        next(filler, None)
                    e2s.append(e2)
                    if debug and b == 0 and qb == 0 and pr == 0:
                        nc.sync.dma_start(
                            dbg_e[:], e2[:].rearrange("p a b -> p (a b)"))
                    # attnV lags scores by one pair so the exp dependency is
                    # already complete (no PE sem-wait stall)
                    if pr >= 1:
                        attnv(pr - 1, False)
                attnv(NPR - 1, True)
                return ops_t

            def normalize(b, qb, ops_t):
                """DVE/gpsimd normalize chain + oT assembly (no PE work).
                Emitted before the next attn_core so psB frees early and oT
                is ready when wo_fill units fire mid-attention."""
                den = smp.tile([1, 1024], f32, tag="den")
                nc.vector.tensor_copy(den[:], ops_t[64:65, :])
                ou = smp.tile([64, 1024], f32, tag="ou")
                nc.vector.tensor_copy(ou[:], ops_t[0:64, :])
                rec = smp.tile([1, 1024], f32, tag="rq")
                nc.vector.reciprocal_approx_fast(rec[:], den[:])
                rb = smp.tile([64, 1024], f32, tag="rb")
                nc.gpsimd.partition_broadcast(rb[:], rec[:])
                oT = osbp.tile([128, QB], bf16, tag="o")
                nc.vector.tensor_tensor(oT[0:64, :], ou[:, 0:512],
                                        rb[0:64, 0:512], op=OP.mult)
                o1 = osbp.tile([64, QB], bf16, tag="o")
                nc.vector.tensor_tensor(o1[:], ou[:, 512:1024],
                                        rb[0:64, 512:1024], op=OP.mult)
                nc.sync.dma_start(oT[64:128, :], o1[:])
                if debug and b == 0 and qb == 0:
                    nc.sync.dma_start(dbg_oT[:], oT[:])
                    nc.sync.dma_start(dbg_rec[:], rec[:])
                return oT

            def batch_tiles(b):
                qT = batchp.tile([128, N], bf16, tag="qT")
                kT = batchp.tile([128, N], bf16, tag="kT")
                v_sb = batchp.tile([128, NKT * 80], edt, tag="v")
                ones = v_sb[:].rearrange(
                    "p (kt c) -> p kt c", c=80)[:, :, 64:65]
                nc.vector.memset(ones, 1.0)
                return (qT, kT, v_sb)

            def chain(*gens):
                for g in gens:
                    yield from g

            def pad(n):
                for _ in range(n):
                    yield

            tiles = batch_tiles(0)
            cur_chunk = load_xT_chunk(0, 0)
            xts_map = {}
            if B > 1:
                xts_map[1] = load_xT(1)
            for c in range(NCH):
                use = cur_chunk
                if c + 1 < NCH:
                    cur_chunk = load_xT_chunk(0, c + 1)
                for _ in proj_fill(0, c, tiles,
                                   lambda et, u=use: u[et][:]):
                    pass
            prev = None  # (b, qb, ops_t) not yet out-projected
            for b in range(B):
                nxt = batch_tiles(b + 1) if b + 1 < B else None
                for i in range(NQB):
                    fills = []
                    if nxt is not None:
                        xb1 = xts_map[b + 1]
                        cs1 = slice(i * CHUNK, (i + 1) * CHUNK)
                        fills.append(proj_fill(
                            b + 1, i, nxt,
                            lambda et, x=xb1, cc=cs1: x[et][:, cc]))
                    else:
                        fills.append(pad(8))
                    if prev is not None:
                        pb, pi, po_t = prev
                        oT_prev = normalize(pb, pi, po_t)
                        fills.append(wo_fill(pb, pi, oT_prev))
                    g = chain(*fills)
                    o = attn_core(b, i, tiles, g)
                    for _ in g:
                        pass
                    prev = (b, i, o)
                if debug and b == 0:
                    nc.sync.dma_start(dbg_qT[:], tiles[0][:])
                    nc.sync.dma_start(dbg_kT[:], tiles[1][:])
                    nc.sync.dma_start(dbg_v[:], tiles[2][:])
                if nxt is not None:
                    tiles = nxt
            pb, pi, po_t = prev
            oT_last = normalize(pb, pi, po_t)
            for _ in wo_fill(pb, pi, oT_last):
                pass

    nc.compile()
    _BUILT[("nc", debug)] = nc
    return nc


def _make_in_maps(x, Wq, Wk, Wv, Wo):
    cos_t, sin_t = _rope_tables()
    x_bf = _to_bf16(np.asarray(x, np.float32))
    in_maps = []
    for d in range(N_CORES):
        wq_d = _to_bf16(np.asarray(Wq, np.float32)[:, d * 128:(d + 1) * 128])
        wk_d = np.asarray(Wk, np.float32)[:, d * 64:(d + 1) * 64]
        wv_d = np.asarray(Wv, np.float32)[:, d * 64:(d + 1) * 64]
        wkv_d = _to_bf16(np.concatenate([wk_d, wv_d], axis=1))
        wo_d = _to_bf16(np.asarray(Wo, np.float32)[d * 128:(d + 1) * 128, :])
        in_maps.append({
            "x": x_bf, "wq": wq_d, "wkv": wkv_d, "wo": wo_d,
            "cos_t": cos_t, "sin_t": sin_t,
        })
    return in_maps


def _run(in_maps, trace=False, trace_kwargs=None, debug=False):
    _ensure_path()
    from concourse.bass_utils import run_bass_kernel_spmd
    nc = _build(debug=debug)
    return run_bass_kernel_spmd(nc, in_maps, list(range(N_CORES)), trace=trace,
                                **(trace_kwargs or {}))


def kernel(x, Wq, Wk, Wv, Wo, bo):
    x = np.asarray(x, dtype=np.float32)
    in_maps = _make_in_maps(np.ascontiguousarray(x.reshape(B, N, DIM)),
                            np.asarray(Wq, np.float32), np.asarray(Wk, np.float32),
                            np.asarray(Wv, np.float32), np.asarray(Wo, np.float32))
    res = _run(in_maps)
    acc = np.zeros((T, DIM), dtype=np.float32)
    for d in range(N_CORES):
        acc += res.results[d]["out"]
    acc += np.asarray(bo, np.float32)[None, :]
    return acc.reshape(B, N, DIM)


# revision 19
# speedup vs baseline: 1.0796x; 1.0796x over previous
"""Trainium2 Bass kernel for nn_Attention_50173807952647.

GQA attention block: qkv projections + partial interleaved RoPE + softmax
attention + output projection, fp32 inputs/outputs.

Sharding: 8 cores; core d owns kv-head d and query heads {2d, 2d+1} for all
4 batches (head/tensor parallel per the GQA grouping). Each core computes a
partial output (its heads' contribution through Wo); host sums partials + bias.

v3: bf16 datapath + XBAR DMA-transpose x^T + fp8e4 DoubleRow attnV +
filler-interleaved emission.
  - Host converts x/weights to bf16; x^T lands in SBUF directly via
    dma_start_transpose (own contiguous [128,512] tile per transfer).
  - Scores: K=64 bf16 matmuls; k^T duplicated once per batch into
    partitions 64:128 so the head-odd matmul pairs with qT[64:128].
  - exp on ACT goes PSUM -> fp8e4 SBUF, pairs of k-tiles share one
    [128,2,1024] tile; attnV contracts both tiles of a pair in a single
    DoubleRow matmul (2 fp8 MACs/cell/cycle). v packed [kt, 80] with the
    denominator ones-column at offset 64.
  - PE queue is fed via a filler generator inside the attention k-tile
    loop: next chunk's projection matmuls first, previous block's Wo
    matmuls after (their oT needs the normalize chain to finish). This
    keeps the PE busy through the exp-gated inner loop (psA double
    buffering caps PE run-ahead at the ACT cadence).
"""

import sys

import numpy as np

HEADS = 16
KV_HEADS = 8
DIM_HEAD = 64
ROT_DIM = 32
SCALE = DIM_HEAD ** -0.5
B, N, DIM = 4, 2048, 1024
N_CORES = 8
T = B * N  # 8192 tokens
CHUNK = 512  # projection chunk (tokens)
QB = 512  # attention query block
FP8_ATTNV = False

_BUILT = {}


def _ensure_path():
    for p in ("/opt/trn_rl_repo",):
        if p not in sys.path:
            sys.path.insert(0, p)


def _to_bf16(a):
    import ml_dtypes
    return np.ascontiguousarray(np.asarray(a, np.float32).astype(ml_dtypes.bfloat16))


def _rope_tables():
    """cos/sin tables [128, N] for the transposed [hd, t] layout.

    Row r (hd index within a core's 128 q-rows): head-local d = r % 64.
    d < ROT_DIM: cos(t * inv_freq[d//2]); sin with rotate-half sign folded
    (-sin on even d, +sin on odd d). Elsewhere cos=1, sin=0 so a single
    full-width mul+add applies RoPE only where it belongs.
    """
    inv_freq = 1.0 / (10000.0 ** (np.arange(0, ROT_DIM, 2, dtype=np.float64) / ROT_DIM))
    t = np.arange(N, dtype=np.float64)
    freqs = t[None, :] * inv_freq[:, None]  # [16, N]
    cos = np.ones((128, N), dtype=np.float64)
    sin = np.zeros((128, N), dtype=np.float64)
    for r in range(128):
        d = r % 64
        if d < ROT_DIM:
            f = freqs[d // 2]
            cos[r] = np.cos(f)
            sin[r] = (-1.0 if d % 2 == 0 else 1.0) * np.sin(f)
    return cos.astype(np.float32), sin.astype(np.float32)


def _build(debug=False):
    if ("nc", debug) in _BUILT:
        return _BUILT[("nc", debug)]
    _ensure_path()
    import concourse.bass as bass  # noqa: F401
    import concourse.mybir as mybir
    import concourse.tile as tile
    from concourse import bacc
    from concourse.masks import make_identity

    dt = mybir.dt
    f32, bf16 = dt.float32, dt.bfloat16
    f8 = dt.float8e4
    edt = f8 if FP8_ATTNV else bf16
    AF = mybir.ActivationFunctionType
    OP = mybir.AluOpType
    PM = mybir.MatmulPerfMode

    nc = bacc.Bacc("TRN2", target_bir_lowering=False, debug=False)

    x_in = nc.dram_tensor("x", [B, N, DIM], bf16, kind="ExternalInput").ap()
    wq_in = nc.dram_tensor("wq", [DIM, 128], bf16, kind="ExternalInput").ap()
    wkv_in = nc.dram_tensor("wkv", [DIM, 128], bf16, kind="ExternalInput").ap()
    wo_in = nc.dram_tensor("wo", [128, DIM], bf16, kind="ExternalInput").ap()
    cos_in = nc.dram_tensor("cos_t", [128, N], f32, kind="ExternalInput").ap()
    sin_in = nc.dram_tensor("sin_t", [128, N], f32, kind="ExternalInput").ap()
    out_d = nc.dram_tensor("out", [T, DIM], f32, kind="ExternalOutput").ap()
    if debug:
        dbg_qT = nc.dram_tensor("dbg_qT", [128, N], bf16, kind="ExternalOutput").ap()
        dbg_kT = nc.dram_tensor("dbg_kT", [128, N], bf16, kind="ExternalOutput").ap()
        dbg_v = nc.dram_tensor("dbg_v", [128, (N // 128) * 80], edt, kind="ExternalOutput").ap()
        dbg_e = nc.dram_tensor("dbg_e", [128, 2048], edt, kind="ExternalOutput").ap()
        dbg_oT = nc.dram_tensor("dbg_oT", [128, QB], bf16, kind="ExternalOutput").ap()
        dbg_rec = nc.dram_tensor("dbg_rec", [1, 1024], f32, kind="ExternalOutput").ap()

    NCH = N // CHUNK  # chunks per batch
    NQB = N // QB  # q blocks per batch
    NKT = N // 128  # key tiles per batch
    NPR = NKT // 2  # key tile pairs
    pair_mask = []
    for i in range(16):
        pair_mask += [2 * i + 1, 2 * i]

    with tile.TileContext(nc) as tc:
        with (
            tc.tile_pool(name="const", bufs=1) as constp,
            tc.tile_pool(name="perbatch", bufs=2) as batchp,
            tc.tile_pool(name="xt", bufs=4) as xtp,
            tc.tile_pool(name="rope", bufs=6) as ropep,
            tc.tile_pool(name="sm", bufs=2) as smp,
            tc.tile_pool(name="exp", bufs=3) as expp,
            tc.tile_pool(name="osb", bufs=4) as osbp,
            tc.tile_pool(name="outsb", bufs=3) as outsbp,
            tc.tile_pool(name="psA", bufs=2, space="PSUM") as psA,
            tc.tile_pool(name="psB", bufs=1, space="PSUM") as psB,
            tc.tile_pool(name="psC", bufs=2, space="PSUM") as psC,
        ):
            ident = constp.tile([128, 128], bf16)
            make_identity(nc, ident[:])
            wq_sb = constp.tile([128, 8 * 128], bf16, tag="wq")
            wkv_sb = constp.tile([128, 8 * 128], bf16, tag="wkv")
            for et in range(8):
                nc.sync.dma_start(wq_sb[:, et * 128:(et + 1) * 128],
                                  wq_in[et * 128:(et + 1) * 128, :])
                nc.sync.dma_start(wkv_sb[:, et * 128:(et + 1) * 128],
                                  wkv_in[et * 128:(et + 1) * 128, :])
            wo_sb = constp.tile([128, DIM], bf16, tag="wo")
            nc.sync.dma_start(wo_sb[:], wo_in[:])
            ebias = constp.tile([128, 1], f32, tag="ebias")
            nc.vector.memset(ebias[:], -2.0 if FP8_ATTNV else 0.0)
            cos_sb = constp.tile([128, N], f32, tag="cos")
            sin_sb = constp.tile([128, N], f32, tag="sin")
            nc.sync.dma_start(cos_sb[:], cos_in[:])
            nc.sync.dma_start(sin_sb[:], sin_in[:])

            def load_xT(b):
                """DMA-transpose a whole batch of x into 8 separate
                [128, N] bf16 tiles (dst of the XBAR transpose must be
                contiguous). Issued a full batch ahead of use so the
                projection filler never waits on DMA arrival."""
                xts = []
                for et in range(8):
                    xte = xtp.tile([128, N], bf16, tag=f"xT{et}")
                    nc.sync.dma_start_transpose(
                        xte[:], x_in[b, :, et * 128:(et + 1) * 128])
                    xts.append(xte)
                return xts

            def load_xT_chunk(b, c):
                """Chunk-sized x^T transposes for the prologue batch (start
                projecting after ~1MB instead of the whole batch)."""
                xts = []
                for et in range(8):
                    xte = xtp.tile([128, CHUNK], bf16, tag=f"xTc{et}")
                    nc.sync.dma_start_transpose(
                        xte[:], x_in[b, c * CHUNK:(c + 1) * CHUNK,
                                     et * 128:(et + 1) * 128])
                    xts.append(xte)
                return xts

            def proj_fill(b, c, tiles, xap):
                """Generator: projection matmuls + rope for chunk c of batch
                b, yielded in PE-sized units so attn_core can interleave.
                xap(et) -> rhs AP of x^T chunk columns for lane et."""
                qT, kT, v_sb = tiles
                cs = slice(c * CHUNK, (c + 1) * CHUNK)
                qps = psC.tile([128, 512], f32, tag="ps_small")
                for et in range(8):
                    nc.tensor.matmul(qps[:],
                                     wq_sb[:, et * 128:(et + 1) * 128],
                                     xap(et),
                                     start=(et == 0), stop=(et == 7))
                    if et % 2 == 1:
                        yield
                kvps = psC.tile([128, 512], f32, tag="ps_small")
                for et in range(8):
                    nc.tensor.matmul(kvps[:],
                                     wkv_sb[:, et * 128:(et + 1) * 128],
                                     xap(et),
                                     start=(et == 0), stop=(et == 7))
                    if et % 2 == 1:
                        yield
                # rope epilogue: q (DVE only)
                shq = ropep.tile([128, CHUNK], f32, tag="rope")
                nc.vector.stream_shuffle(shq[:], qps[:], pair_mask)
                t1q = ropep.tile([128, CHUNK], f32, tag="rope")
                nc.vector.tensor_tensor(t1q[:], qps[:], cos_sb[:, cs], op=OP.mult)
                t2q = ropep.tile([128, CHUNK], f32, tag="rope")
                nc.vector.tensor_tensor(t2q[:], shq[:], sin_sb[:, cs], op=OP.mult)
                nc.vector.tensor_tensor(qT[:, cs], t1q[:], t2q[:], op=OP.add)
                yield
                # rope epilogue: k -> kT rows 0:64 (DVE only)
                shk = ropep.tile([32, CHUNK], f32, tag="rope")
                nc.vector.stream_shuffle(shk[:], kvps[0:32, :], pair_mask)
                t1k = ropep.tile([64, CHUNK], f32, tag="rope")
                nc.vector.tensor_tensor(t1k[:], kvps[0:64, :], cos_sb[0:64, cs],
                                        op=OP.mult)
                t2k = ropep.tile([32, CHUNK], f32, tag="rope")
                nc.vector.tensor_tensor(t2k[:], shk[:], sin_sb[0:32, cs], op=OP.mult)
                nc.vector.tensor_tensor(kT[0:32, cs], t1k[0:32, :], t2k[:], op=OP.add)
                nc.vector.tensor_copy(kT[32:64, cs], t1k[32:64, :])
                # duplicate k^T into partitions 64:128 so the head-odd score
                # matmul can pair with qT[64:128] (matmul needs equal base
                # partitions). Issued via SWDGE (gpsimd): an SBUF->SBUF copy
                # on the sync HWDGE queue serializes against the x-transpose
                # stream (transpose||SBUF-SBUF hazard) and stalls the scores.
                nc.gpsimd.dma_start(kT[64:128, cs], kT[0:64, cs])
                yield
                # v staging copy (DVE), then PE transposes + pack
                vts = ropep.tile([64, CHUNK], bf16, tag="ropev")
                nc.vector.tensor_copy(vts[:], kvps[64:128, :])
                yield
                vtp = psC.tile([128, 512], bf16, tag="ps_small")
                for st in range(4):
                    nc.tensor.transpose(vtp[:, st * 128: st * 128 + 64],
                                        vts[:, st * 128:(st + 1) * 128],
                                        ident[0:64, 0:64])
                yield
                v3 = v_sb[:].rearrange("p (kt c) -> p kt c", c=80)
                for st in range(4):
                    kt = c * 4 + st
                    nc.vector.tensor_copy(v3[:, kt, 0:64],
                                          vtp[:, st * 128: st * 128 + 64])
                yield

            def wo_fill(b, qb, oT):
                """Generator: previous block's out-projection, one matmul +
                staging copy + DMA per unit."""
                for ts in range(4):
                    for eh in range(2):
                        po = psC.tile([128, 512], f32, tag="ps_small")
                        nc.tensor.matmul(po[:],
                                         oT[:, ts * 128:(ts + 1) * 128],
                                         wo_sb[:, eh * 512:(eh + 1) * 512],
                                         start=True, stop=True)
                        ob = outsbp.tile([128, 512], f32, tag="ob")
                        nc.vector.tensor_copy(ob[:], po[:])
                        r0 = b * N + qb * QB + ts * 128
                        nc.sync.dma_start(
                            out_d[r0:r0 + 128, eh * 512:(eh + 1) * 512], ob[:])
                        yield

            def attn_core(b, qb, tiles, filler):
                qT, kT, v_sb = tiles
                v3 = v_sb[:].rearrange("p (kt c) -> p kt c", c=80)
                qs = slice(qb * QB, (qb + 1) * QB)
                ops_t = psB.tile([65, 1024], f32, tag="ps_o")

                def attnv(j, last):
                    if FP8_ATTNV:
                        nc.tensor.matmul(ops_t[:, 0:512],
                                         v3[:, 2 * j:2 * j + 2, 0:65],
                                         e2s[j][:, :, 0:512],
                                         start=(j == 0), stop=last,
                                         perf_mode=PM.DoubleRow)
                        nc.tensor.matmul(ops_t[:, 512:1024],
                                         v3[:, 2 * j:2 * j + 2, 0:65],
                                         e2s[j][:, :, 512:1024],
                                         start=(j == 0), stop=last,
                                         perf_mode=PM.DoubleRow)
                    else:
                        for h in range(2):
                            for kt in (2 * j, 2 * j + 1):
                                nc.tensor.matmul(
                                    ops_t[:, h * 512:(h + 1) * 512],
                                    v3[:, kt, 0:65],
                                    e2s[j][:, kt - 2 * j, h * 512:(h + 1) * 512],
                                    start=(kt == 0 and j == 0),
                                    stop=(last and kt == 2 * j + 1))

                e2s = []
                for pr in range(NPR):
                    e2 = expp.tile([128, 2, 1024], edt, tag="e")
                    for half in range(2):
                        kt = 2 * pr + half
                        ks = slice(kt * 128, (kt + 1) * 128)
                        sps = psA.tile([128, 1024], f32, tag="ps_big")
                        nc.tensor.matmul(sps[:, 0:512], kT[0:64, ks],
                                         qT[0:64, qs], start=True, stop=True)
                        nc.tensor.matmul(sps[:, 512:1024], kT[64:128, ks],
                                         qT[64:128, qs], start=True, stop=True)
                        # bias -2 keeps exp below the fp8e4 max (~240) for
                        # up to ~7.5-sigma scores; softmax shift-invariance
                        # cancels it exactly in O/den
                        nc.scalar.activation(e2[:, half, :], sps[:],
                                             AF.Exp, scale=SCALE,
                                             bias=ebias[:])
                        next(filler, None)
                    e2s.append(e2)
                    if debug and b == 0 and qb == 0 and pr == 0:
                        nc.sync.dma_start(
                            dbg_e[:], e2[:].rearrange("p a b -> p (a b)"))
                    # attnV lags scores by one pair so the exp dependency is
                    # already complete (no PE sem-wait stall)
                    if pr >= 1:
                        attnv(pr - 1, False)
                attnv(NPR - 1, True)
                return ops_t

            def normalize(b, qb, ops_t):
                """DVE/gpsimd normalize chain + oT assembly (no PE work).
                Emitted before the next attn_core so psB frees early and oT
                is ready when wo_fill units fire mid-attention."""
                den = smp.tile([1, 1024], f32, tag="den")
                nc.vector.tensor_copy(den[:], ops_t[64:65, :])
                ou = smp.tile([64, 1024], f32, tag="ou")
                nc.vector.tensor_copy(ou[:], ops_t[0:64, :])
                rec = smp.tile([1, 1024], f32, tag="rq")
                nc.vector.reciprocal_approx_fast(rec[:], den[:])
                rb = smp.tile([64, 1024], f32, tag="rb")
                nc.gpsimd.partition_broadcast(rb[:], rec[:])
                oT = osbp.tile([128, QB], bf16, tag="o")
                nc.vector.tensor_tensor(oT[0:64, :], ou[:, 0:512],
                                        rb[0:64, 0:512], op=OP.mult)
                o1 = osbp.tile([64, QB], bf16, tag="o")
                nc.vector.tensor_tensor(o1[:], ou[:, 512:1024],
                                        rb[0:64, 512:1024], op=OP.mult)
                nc.gpsimd.dma_start(oT[64:128, :], o1[:])
                if debug and b == 0 and qb == 0:
                    nc.sync.dma_start(dbg_oT[:], oT[:])
                    nc.sync.dma_start(dbg_rec[:], rec[:])
                return oT

            def batch_tiles(b):
                qT = batchp.tile([128, N], bf16, tag="qT")
                kT = batchp.tile([128, N], bf16, tag="kT")
                v_sb = batchp.tile([128, NKT * 80], edt, tag="v")
                ones = v_sb[:].rearrange(
                    "p (kt c) -> p kt c", c=80)[:, :, 64:65]
                nc.vector.memset(ones, 1.0)
                return (qT, kT, v_sb)

            def chain(*gens):
                for g in gens:
                    yield from g

            def pad(n):
                for _ in range(n):
                    yield

            tiles = batch_tiles(0)
            cur_chunk = load_xT_chunk(0, 0)
            xts_map = {}
            if B > 1:
                xts_map[1] = load_xT(1)
            for c in range(NCH):
                use = cur_chunk
                if c + 1 < NCH:
                    cur_chunk = load_xT_chunk(0, c + 1)
                for _ in proj_fill(0, c, tiles,
                                   lambda et, u=use: u[et][:]):
                    pass
            prev = None  # (b, qb, ops_t) not yet out-projected
            for b in range(B):
                nxt = batch_tiles(b + 1) if b + 1 < B else None
                for i in range(NQB):
                    fills = []
                    if nxt is not None:
                        xb1 = xts_map[b + 1]
                        cs1 = slice(i * CHUNK, (i + 1) * CHUNK)
                        fills.append(proj_fill(
                            b + 1, i, nxt,
                            lambda et, x=xb1, cc=cs1: x[et][:, cc]))
                    else:
                        fills.append(pad(8))
                    if prev is not None:
                        pb, pi, po_t = prev
                        oT_prev = normalize(pb, pi, po_t)
                        fills.append(wo_fill(pb, pi, oT_prev))
                    g = chain(*fills)
                    o = attn_core(b, i, tiles, g)
                    for _ in g:
                        pass
                    prev = (b, i, o)
                if debug and b == 0:
                    nc.sync.dma_start(dbg_qT[:], tiles[0][:])
                    nc.sync.dma_start(dbg_kT[:], tiles[1][:])
                    nc.sync.dma_start(dbg_v[:], tiles[2][:])
                if nxt is not None:
                    tiles = nxt
            pb, pi, po_t = prev
            oT_last = normalize(pb, pi, po_t)
            for _ in wo_fill(pb, pi, oT_last):
                pass

    nc.compile()
    _BUILT[("nc", debug)] = nc
    return nc


def _make_in_maps(x, Wq, Wk, Wv, Wo):
    cos_t, sin_t = _rope_tables()
    x_bf = _to_bf16(np.asarray(x, np.float32))
    in_maps = []
    for d in range(N_CORES):
        wq_d = _to_bf16(np.asarray(Wq, np.float32)[:, d * 128:(d + 1) * 128])
        wk_d = np.asarray(Wk, np.float32)[:, d * 64:(d + 1) * 64]
        wv_d = np.asarray(Wv, np.float32)[:, d * 64:(d + 1) * 64]
        wkv_d = _to_bf16(np.concatenate([wk_d, wv_d], axis=1))
        wo_d = _to_bf16(np.asarray(Wo, np.float32)[d * 128:(d + 1) * 128, :])
        in_maps.append({
            "x": x_bf, "wq": wq_d, "wkv": wkv_d, "wo": wo_d,
            "cos_t": cos_t, "sin_t": sin_t,
        })
    return in_maps


def _run(in_maps, trace=False, trace_kwargs=None, debug=False):
    _ensure_path()
    from concourse.bass_utils import run_bass_kernel_spmd
    nc = _build(debug=debug)
    return run_bass_kernel_spmd(nc, in_maps, list(range(N_CORES)), trace=trace,
                                **(trace_kwargs or {}))


def kernel(x, Wq, Wk, Wv, Wo, bo):
    x = np.asarray(x, dtype=np.float32)
    in_maps = _make_in_maps(np.ascontiguousarray(x.reshape(B, N, DIM)),
                            np.asarray(Wq, np.float32), np.asarray(Wk, np.float32),
                            np.asarray(Wv, np.float32), np.asarray(Wo, np.float32))
    res = _run(in_maps)
    acc = np.zeros((T, DIM), dtype=np.float32)
    for d in range(N_CORES):
        acc += res.results[d]["out"]
    acc += np.asarray(bo, np.float32)[None, :]
    return acc.reshape(B, N, DIM)
